# revision 24
# baseline (speedup 1.0000x reference)
"""SGC graph-conv kernel for Trainium2 (8 NeuronCores, SPMD).

Computes: out = segment_sum(edge_val[:,None] * feat[edge_col], edge_row) @ W.T + b

Strategy: 1D row-partition by destination (edge_row is sorted, so each core's
edges are a contiguous slice). Each core owns 12500 destination rows, padded
to 12600 = 420 blocks of R=30 rows, processed in 14 supers of 30 blocks.

Gather: indexed DMA descriptors cost ~3ns each (SWDGE gen + queue drain, 4
parallel SWDGE queues), so descriptor COUNT is the wall. Batched gathers use
InstDMAGatherAnt (dma_gather), whose int16 indices reach only 32768 rows;
feat is split into 4 static chunks of 25000 rows, one gather call per
(super, chunk) on its own SWDGE queue. Each (block, chunk) bucket gets one
128-slot tile (mean ~119 edges = 93% fill); bucket overflow spills into one
shared per-(super, chunk) overflow tile appended to the same gather call
(max observed spill 123 <= 128) and is routed by a super-wide one-hot, so
no extra DMA instructions are needed. Pad slots re-fetch chunk row 0
(valid, row-buffer hot) and are zeroed via S.

Per super: 4 dma_gather calls of 3968 rows each (30 grid tiles + 1 overflow
tile, chunk-major into contiguous SBUF) -> m (f32) -> ACT copy to bf16.
S one-hot built in 4 batched DVE ops with stride-0 broadcast APs:
    eq = (lrow == iota)   s = eq * val      (pads ship lrow=1e6 -> s=0)
Each block accumulates hT[64, 30] in PSUM over its 4 chunk tiles + the 4
overflow tiles (auto-masked by is_equal against iota900). Then per 3-block
hT group: ACT copy to bf16, matmul vs W.T (bf16), bias add during the
PSUM->SBUF copy, one batched 900-row output DMA per super. The padded
output rows (12500..12599) are dropped on the host.
"""

import sys

sys.path.insert(0, "/opt/trn_rl_repo")

import numpy as np

N_NODES = 100000
N_EDGES = 1600000
F = 64
C = 64
N_CORES = 8
ROWS_PER_CORE = N_NODES // N_CORES  # 12500
P = 128
R = 30                      # rows per block
RPC_PAD = 12600             # padded rows per core (= 420 * 30)
NB = RPC_PAD // R           # 420 blocks per core
NCHUNK = 4
CHUNK = N_NODES // NCHUNK   # 25000 (< 32768: int16-addressable)
SPB = 30                    # blocks per super
NSUP = NB // SPB            # 14 supers per core
TPC = SPB + 1               # tiles per (super, chunk) segment: 30 grid + 1 ov
TPS = NCHUNK * TPC          # 124 tiles per super
NTILE = NSUP * TPS          # tiles per core
SROWS = SPB * R             # 900 rows per super
HGB = 3                     # blocks per hT PSUM group
IDXC = TPC * P // 16        # idx16 columns per gather call (248)

_CACHE = {}


def _build_program():
    from concourse import bacc, bass, mybir
    from concourse.ap import AP
    from concourse.tile import TileContext

    f32 = mybir.dt.float32
    bf16 = mybir.dt.bfloat16
    i16 = mybir.dt.int16
    Copy = mybir.ActivationFunctionType.Copy
    Alu = mybir.AluOpType

    GRID = NCHUNK * SPB          # 120 grid tiles per super
    SW = GRID * R + NCHUNK * SROWS  # s columns per super (grid + 4 ov)
    NHG = SPB // HGB             # 10 hT groups per super
    HR = HGB * R                 # 90 rows per hT group

    # meta layout (per super: 120 grid lrow/vals cols then 4 ov cols)
    W_META = 2 * NTILE + R + SROWS + C + C
    OFF_LROW = 0
    OFF_VALS = NTILE
    OFF_IR = 2 * NTILE
    OFF_IS = 2 * NTILE + R
    OFF_WT = 2 * NTILE + R + SROWS
    OFF_B = OFF_WT + C

    nc = bacc.Bacc(num_swdge_queues=4)
    feat_d = nc.dram_tensor("feat", [N_NODES, F], f32, kind="ExternalInput")
    idx_d = nc.dram_tensor("idx16", [P, NSUP * NCHUNK * IDXC], i16,
                           kind="ExternalInput")
    meta_d = nc.dram_tensor("meta", [P, W_META], f32, kind="ExternalInput")
    out_d = nc.dram_tensor("out", [RPC_PAD, C], f32, kind="ExternalOutput")

    with TileContext(nc) as tc:
        with (
            tc.tile_pool(name="const", bufs=1) as cpool,
            tc.tile_pool(name="m", bufs=2) as mpool,
            tc.tile_pool(name="mb", bufs=2) as mbpool,
            tc.tile_pool(name="u", bufs=2) as upool,
            tc.tile_pool(name="s", bufs=2) as spool,
            tc.tile_pool(name="sov", bufs=2) as sovpool,
            tc.tile_pool(name="hts", bufs=3) as htspool,
            tc.tile_pool(name="osb", bufs=2) as opool,
            tc.tile_pool(name="psum_h", bufs=2, space="PSUM") as hpsum,
            tc.tile_pool(name="psum_o", bufs=2, space="PSUM") as opsum,
        ):
            meta_sb = cpool.tile([P, W_META], f32)
            wt_bf = cpool.tile([F, C], bf16)
            nc.sync.dma_start(out=meta_sb[:], in_=meta_d[:])
            nc.scalar.activation(
                out=wt_bf[:], in_=meta_sb[:F, OFF_WT : OFF_WT + C], func=Copy
            )
            brep_ap = meta_sb[:HR, OFF_B : OFF_B + C]

            for sp in range(NSUP):
                it = cpool.tile([P, NCHUNK * IDXC], i16, tag=f"idx{sp}")
                nc.sync.dma_start(
                    out=it[:],
                    in_=idx_d[:, sp * NCHUNK * IDXC : (sp + 1) * NCHUNK * IDXC],
                )
                t0 = sp * TPS
                mbs = []
                sgs = []
                sovs = []
                for q in range(NCHUNK):
                    m = mpool.tile([P, TPC * F], f32, tag=f"m{q}")
                    m_ap = m[:]
                    out3d = AP(
                        m_ap.tensor,
                        m_ap.offset,
                        [m_ap.ap[0], [F, TPC], [1, F]],
                    )
                    nc.gpsimd.dma_gather(
                        out_ap=out3d,
                        in_ap=feat_d[q * CHUNK : (q + 1) * CHUNK],
                        idxs_ap=it[:, q * IDXC : (q + 1) * IDXC],
                        num_idxs=TPC * P,
                        num_idxs_reg=TPC * P,
                        elem_size=F,
                        single_packet=False,
                        queue_num=q,
                    )
                    mb = mbpool.tile([P, TPC * F], bf16, tag=f"mb{q}")
                    nc.scalar.activation(out=mb[:], in_=m[:], func=Copy)
                    mbs.append(mb)

                    u = upool.tile([P, SPB * R], bf16, tag=f"u{q}")
                    sg = spool.tile([P, SPB * R], bf16, tag=f"s{q}")
                    lrow_g = (
                        meta_sb[:, OFF_LROW + t0 + q * SPB : OFF_LROW + t0 + (q + 1) * SPB]
                        .unsqueeze(2)
                        .broadcast_to([P, SPB, R])
                    )
                    vals_g = (
                        meta_sb[:, OFF_VALS + t0 + q * SPB : OFF_VALS + t0 + (q + 1) * SPB]
                        .unsqueeze(2)
                        .broadcast_to([P, SPB, R])
                    )
                    iota_g = (
                        meta_sb[:, OFF_IR : OFF_IR + R]
                        .unsqueeze(1)
                        .broadcast_to([P, SPB, R])
                    )
                    nc.vector.scalar_tensor_tensor(
                        out=u[:], in0=lrow_g, scalar=0.0, in1=iota_g,
                        op0=Alu.bypass, op1=Alu.is_equal,
                    )
                    nc.vector.scalar_tensor_tensor(
                        out=sg[:], in0=u[:], scalar=0.0,
                        in1=vals_g, op0=Alu.bypass, op1=Alu.mult,
                    )
                    sgs.append(sg)

                    uo = upool.tile([P, SROWS], bf16, tag=f"uo{q}")
                    sov = sovpool.tile([P, SROWS], bf16, tag=f"sov{q}")
                    lrow_o = (
                        meta_sb[:, OFF_LROW + t0 + GRID + q : OFF_LROW + t0 + GRID + q + 1]
                        .unsqueeze(2)
                        .broadcast_to([P, 1, SROWS])
                    )
                    vals_o = (
                        meta_sb[:, OFF_VALS + t0 + GRID + q : OFF_VALS + t0 + GRID + q + 1]
                        .unsqueeze(2)
                        .broadcast_to([P, 1, SROWS])
                    )
                    iota_o = (
                        meta_sb[:, OFF_IS : OFF_IS + SROWS]
                        .unsqueeze(1)
                        .broadcast_to([P, 1, SROWS])
                    )
                    nc.vector.scalar_tensor_tensor(
                        out=uo[:], in0=lrow_o, scalar=0.0, in1=iota_o,
                        op0=Alu.bypass, op1=Alu.is_equal,
                    )
                    nc.vector.scalar_tensor_tensor(
                        out=sov[:], in0=uo[:], scalar=0.0,
                        in1=vals_o, op0=Alu.bypass, op1=Alu.mult,
                    )
                    sovs.append(sov)

                osb = opool.tile([HR, NHG * C], f32, tag="osb")
                for hg in range(NHG):
                    hT = hpsum.tile([F, HR], f32, tag="hT")
                    for b3 in range(HGB):
                        b = hg * HGB + b3
                        for q in range(NCHUNK):
                            nc.tensor.matmul(
                                out=hT[:, b3 * R : (b3 + 1) * R],
                                lhsT=mbs[q][:, b * F : (b + 1) * F],
                                rhs=sgs[q][:, b * R : (b + 1) * R],
                                start=(q == 0),
                                stop=False,
                            )
                        for q in range(NCHUNK):
                            nc.tensor.matmul(
                                out=hT[:, b3 * R : (b3 + 1) * R],
                                lhsT=mbs[q][:, SPB * F : TPC * F],
                                rhs=sovs[q][:, b * R : (b + 1) * R],
                                start=False,
                                stop=(q == NCHUNK - 1),
                            )
                    hts = htspool.tile([F, HR], bf16, tag="hts")
                    nc.scalar.activation(out=hts[:], in_=hT[:], func=Copy)
                    o_ps = opsum.tile([HR, C], f32, tag="o")
                    nc.tensor.matmul(
                        out=o_ps[:], lhsT=hts[:], rhs=wt_bf[:],
                        start=True, stop=True,
                    )
                    nc.vector.tensor_add(
                        out=osb[:, hg * C : (hg + 1) * C], in0=o_ps[:], in1=brep_ap
                    )
                ob = out_d[:]
                out_ap = AP(
                    ob.tensor,
                    sp * SROWS * C,
                    [[C, HR], [HR * C, NHG], [1, C]],
                )
                nc.sync.dma_start(out=out_ap, in_=osb[:])

    if not nc.is_finalized():
        nc.finalize()
    return nc


def _prep(feat, edge_row, edge_col, edge_val, W, b):
    """Host-side packing for the chunked-gather layout. See module docstring."""
    feat = np.ascontiguousarray(np.asarray(feat, dtype=np.float32))
    er = np.asarray(edge_row, dtype=np.int64)
    ec = np.asarray(edge_col, dtype=np.int64)
    ev = np.asarray(edge_val, dtype=np.float32)
    W = np.asarray(W, dtype=np.float32)
    b = np.asarray(b, dtype=np.float32)

    core = er // ROWS_PER_CORE          # 0..7
    lr = er % ROWS_PER_CORE             # core-local row
    blk = lr // R                       # block within core 0..419
    q = ec // CHUNK                     # chunk 0..3
    pk = (core * NB + blk) * NCHUNK + q
    order = np.lexsort((ec, pk))
    ecol = ec[order]
    evv = ev[order]
    pks = pk[order]
    lrs = lr[order]
    eqs = q[order]

    nbuk = N_CORES * NB * NCHUNK
    counts = np.bincount(pks, minlength=nbuk)
    starts = np.concatenate([[0], np.cumsum(counts)])
    rank = np.arange(N_EDGES) - starts[pks]

    cores = pks // (NB * NCHUNK)
    blks = (pks // NCHUNK) % NB
    sps = blks // SPB
    bls = blks % SPB

    GRID = NCHUNK * SPB
    NIDX = NSUP * NCHUNK * IDXC
    idx16 = np.zeros((N_CORES, 16, NIDX), dtype=np.int16)  # pad = chunk row 0
    lrow = np.full((N_CORES, P, NTILE), 1.0e6, dtype=np.float32)
    vals = np.zeros((N_CORES, P, NTILE), dtype=np.float32)

    grid = rank < P
    g_rank = rank[grid]
    slot = bls[grid] * P + g_rank
    colpos = (sps[grid] * NCHUNK + eqs[grid]) * IDXC + slot // 16
    idx16[cores[grid], slot % 16, colpos] = (
        ecol[grid] - eqs[grid] * CHUNK
    ).astype(np.int16)
    g_tile = sps[grid] * TPS + eqs[grid] * SPB + bls[grid]  # meta: grid-first
    lrow[cores[grid], g_rank, g_tile] = (lrs[grid] % R).astype(np.float32)
    vals[cores[grid], g_rank, g_tile] = evv[grid]

    # ---- overflow placement: per (core, super, chunk) shared tile ----
    ovf = ~grid
    if ovf.any():
        okey = (cores[ovf] * NSUP + sps[ovf]) * NCHUNK + eqs[ovf]
        oorder = np.argsort(okey, kind="stable")
        okey_s = okey[oorder]
        ocnt = np.bincount(okey_s, minlength=N_CORES * NSUP * NCHUNK)
        if ocnt.max() > P:
            raise RuntimeError(f"overflow tile overflow: {ocnt.max()} > {P}")
        ostarts = np.concatenate([[0], np.cumsum(ocnt)])
        okk = np.arange(okey_s.size) - ostarts[okey_s]
        oc = cores[ovf][oorder]
        osp = sps[ovf][oorder]
        oq = eqs[ovf][oorder]
        oslot = SPB * P + okk                     # within the (sp, q) call
        ocolpos = (osp * NCHUNK + oq) * IDXC + oslot // 16
        idx16[oc, oslot % 16, ocolpos] = (
            ecol[ovf][oorder] - oq * CHUNK
        ).astype(np.int16)
        o_tile = osp * TPS + GRID + oq            # meta: ov cols after grid
        lrow[oc, okk, o_tile] = (lrs[ovf][oorder] - osp * SROWS).astype(
            np.float32
        )
        vals[oc, okk, o_tile] = evv[ovf][oorder]

    iotar = np.tile(np.arange(R, dtype=np.float32)[None, :], (P, 1))
    iotas = np.tile(np.arange(SROWS, dtype=np.float32)[None, :], (P, 1))
    wt_pad = np.zeros((P, C), dtype=np.float32)
    wt_pad[:F, :] = W.T
    brep = np.tile(b[None, :], (P, 1)).astype(np.float32)

    in_maps = []
    for c in range(N_CORES):
        meta = np.concatenate(
            [lrow[c], vals[c], iotar, iotas, wt_pad, brep], axis=1
        )
        in_maps.append(
            {
                "feat": feat,
                "idx16": np.ascontiguousarray(np.tile(idx16[c], (8, 1))),
                "meta": np.ascontiguousarray(meta),
            }
        )
    return in_maps


def kernel(feat, edge_row, edge_col, edge_val, W, b, _trace=False, _trace_kwargs=None):
    from concourse.bass_utils import run_bass_kernel_spmd

    in_maps = _prep(feat, edge_row, edge_col, edge_val, W, b)
    if "prog" not in _CACHE:
        _CACHE["prog"] = _build_program()
    nc = _CACHE["prog"]
    kw = {}
    if _trace:
        kw["trace"] = True
        kw.update(_trace_kwargs or {})
    res = run_bass_kernel_spmd(nc, in_maps, list(range(N_CORES)), **kw)
    out = np.concatenate([r["out"][:ROWS_PER_CORE] for r in res.results], axis=0)
    if _trace:
        return out, res
    return out


# revision 25
# speedup vs baseline: 1.0286x; 1.0286x over previous
"""SGC graph-conv kernel for Trainium2 (8 NeuronCores, SPMD).

Computes: out = segment_sum(edge_val[:,None] * feat[edge_col], edge_row) @ W.T + b

Strategy: 1D row-partition by destination (edge_row is sorted, so each core's
edges are a contiguous slice). Each core owns 12500 destination rows, padded
to 12600 = 420 blocks of R=30 rows, processed in 14 supers of 30 blocks.

Gather: indexed DMA descriptors cost ~3ns each (SWDGE gen + queue drain, 4
parallel SWDGE queues), so descriptor COUNT is the wall. Batched gathers use
InstDMAGatherAnt (dma_gather), whose int16 indices reach only 32768 rows;
feat is split into 4 static chunks of 25000 rows, one gather call per
(super, chunk) on its own SWDGE queue. Each (block, chunk) bucket gets one
128-slot tile (mean ~119 edges = 93% fill); bucket overflow spills into one
shared per-(super, chunk) overflow tile appended to the same gather call
(max observed spill 123 <= 128) and is routed by a super-wide one-hot, so
no extra DMA instructions are needed. Pad slots re-fetch chunk row 0
(valid, row-buffer hot) and are zeroed via S.

Per super: 4 dma_gather calls of 3968 rows each (30 grid tiles + 1 overflow
tile, chunk-major into contiguous SBUF) -> m (f32) -> ACT copy to bf16.
S one-hot built in 4 batched DVE ops with stride-0 broadcast APs:
    eq = (lrow == iota)   s = eq * val      (pads ship lrow=1e6 -> s=0)
Each block accumulates hT[64, 30] in PSUM over its 4 chunk tiles + the 4
overflow tiles (auto-masked by is_equal against iota900). Then per 3-block
hT group: ACT copy to bf16, matmul vs W.T (bf16), bias add during the
PSUM->SBUF copy, one batched 900-row output DMA per super. The padded
output rows (12500..12599) are dropped on the host.
"""

import sys

sys.path.insert(0, "/opt/trn_rl_repo")

import numpy as np

N_NODES = 100000
N_EDGES = 1600000
F = 64
C = 64
N_CORES = 8
ROWS_PER_CORE = N_NODES // N_CORES  # 12500
P = 128
R = 30                      # rows per block
RPC_PAD = 12600             # padded rows per core (= 420 * 30)
NB = RPC_PAD // R           # 420 blocks per core
NCHUNK = 4
CHUNK = N_NODES // NCHUNK   # 25000 (< 32768: int16-addressable)
SPB = 30                    # blocks per super
NSUP = NB // SPB            # 14 supers per core
TPC = SPB + 1               # tiles per (super, chunk) segment: 30 grid + 1 ov
TPS = NCHUNK * TPC          # 124 tiles per super
NTILE = NSUP * TPS          # tiles per core
SROWS = SPB * R             # 900 rows per super
HGB = 3                     # blocks per hT PSUM group
IDXC = TPC * P // 16        # idx16 columns per gather call (248)

_CACHE = {}


def _build_program():
    from concourse import bacc, bass, mybir
    from concourse.ap import AP
    from concourse.tile import TileContext

    f32 = mybir.dt.float32
    bf16 = mybir.dt.bfloat16
    i16 = mybir.dt.int16
    Copy = mybir.ActivationFunctionType.Copy
    Alu = mybir.AluOpType

    GRID = NCHUNK * SPB          # 120 grid tiles per super
    SW = GRID * R + NCHUNK * SROWS  # s columns per super (grid + 4 ov)
    NHG = SPB // HGB             # 10 hT groups per super
    HR = HGB * R                 # 90 rows per hT group

    # meta layout (per super: 120 grid lrow/vals cols then 4 ov cols)
    W_META = 2 * NTILE + R + SROWS + C + C
    OFF_LROW = 0
    OFF_VALS = NTILE
    OFF_IR = 2 * NTILE
    OFF_IS = 2 * NTILE + R
    OFF_WT = 2 * NTILE + R + SROWS
    OFF_B = OFF_WT + C

    nc = bacc.Bacc(num_swdge_queues=4)
    feat_d = nc.dram_tensor("feat", [N_NODES, F], f32, kind="ExternalInput")
    idx_d = nc.dram_tensor("idx16", [P, NSUP * NCHUNK * IDXC], i16,
                           kind="ExternalInput")
    meta_d = nc.dram_tensor("meta", [P, W_META], f32, kind="ExternalInput")
    out_d = nc.dram_tensor("out", [RPC_PAD, C], f32, kind="ExternalOutput")

    with TileContext(nc) as tc:
        with (
            tc.tile_pool(name="const", bufs=1) as cpool,
            tc.tile_pool(name="m", bufs=2) as mpool,
            tc.tile_pool(name="mb", bufs=2) as mbpool,
            tc.tile_pool(name="u", bufs=2) as upool,
            tc.tile_pool(name="s", bufs=2) as spool,
            tc.tile_pool(name="hts", bufs=3) as htspool,
            tc.tile_pool(name="osb", bufs=2) as opool,
            tc.tile_pool(name="psum_h", bufs=2, space="PSUM") as hpsum,
            tc.tile_pool(name="psum_o", bufs=2, space="PSUM") as opsum,
        ):
            meta_sb = cpool.tile([P, W_META], f32)
            wt_bf = cpool.tile([F, C], bf16)
            idx_tiles = []
            for sp in range(NSUP):
                it = cpool.tile([P, NCHUNK * IDXC], i16, tag=f"idx{sp}")
                nc.sync.dma_start(
                    out=it[:],
                    in_=idx_d[:, sp * NCHUNK * IDXC : (sp + 1) * NCHUNK * IDXC],
                )
                idx_tiles.append(it)
            nc.sync.dma_start(out=meta_sb[:], in_=meta_d[:])
            nc.scalar.activation(
                out=wt_bf[:], in_=meta_sb[:F, OFF_WT : OFF_WT + C], func=Copy
            )
            brep_ap = meta_sb[:HR, OFF_B : OFF_B + C]

            for sp in range(NSUP):
                m = mpool.tile([P, TPS * F], f32, tag="m")
                m_ap = m[:]
                for q in range(NCHUNK):
                    out3d = AP(
                        m_ap.tensor,
                        m_ap.offset + q * TPC * F,
                        [m_ap.ap[0], [F, TPC], [1, F]],
                    )
                    nc.gpsimd.dma_gather(
                        out_ap=out3d,
                        in_ap=feat_d[q * CHUNK : (q + 1) * CHUNK],
                        idxs_ap=idx_tiles[sp][:, q * IDXC : (q + 1) * IDXC],
                        num_idxs=TPC * P,
                        num_idxs_reg=TPC * P,
                        elem_size=F,
                        single_packet=False,
                        queue_num=q,
                    )
                mb = mbpool.tile([P, TPS * F], bf16, tag="mb")
                nc.scalar.activation(out=mb[:], in_=m[:], func=Copy)

                u = upool.tile([P, SW], bf16, tag="u")
                s = spool.tile([P, SW], bf16, tag="s")
                t0 = sp * TPS
                lrow_g = (
                    meta_sb[:, OFF_LROW + t0 : OFF_LROW + t0 + GRID]
                    .unsqueeze(2)
                    .broadcast_to([P, GRID, R])
                )
                vals_g = (
                    meta_sb[:, OFF_VALS + t0 : OFF_VALS + t0 + GRID]
                    .unsqueeze(2)
                    .broadcast_to([P, GRID, R])
                )
                iota_g = (
                    meta_sb[:, OFF_IR : OFF_IR + R]
                    .unsqueeze(1)
                    .broadcast_to([P, GRID, R])
                )
                nc.vector.scalar_tensor_tensor(
                    out=u[:, : GRID * R], in0=lrow_g, scalar=0.0, in1=iota_g,
                    op0=Alu.bypass, op1=Alu.is_equal,
                )
                nc.vector.scalar_tensor_tensor(
                    out=s[:, : GRID * R], in0=u[:, : GRID * R], scalar=0.0,
                    in1=vals_g, op0=Alu.bypass, op1=Alu.mult,
                )
                lrow_o = (
                    meta_sb[:, OFF_LROW + t0 + GRID : OFF_LROW + t0 + TPS]
                    .unsqueeze(2)
                    .broadcast_to([P, NCHUNK, SROWS])
                )
                vals_o = (
                    meta_sb[:, OFF_VALS + t0 + GRID : OFF_VALS + t0 + TPS]
                    .unsqueeze(2)
                    .broadcast_to([P, NCHUNK, SROWS])
                )
                iota_o = (
                    meta_sb[:, OFF_IS : OFF_IS + SROWS]
                    .unsqueeze(1)
                    .broadcast_to([P, NCHUNK, SROWS])
                )
                nc.vector.scalar_tensor_tensor(
                    out=u[:, GRID * R :], in0=lrow_o, scalar=0.0, in1=iota_o,
                    op0=Alu.bypass, op1=Alu.is_equal,
                )
                nc.vector.scalar_tensor_tensor(
                    out=s[:, GRID * R :], in0=u[:, GRID * R :], scalar=0.0,
                    in1=vals_o, op0=Alu.bypass, op1=Alu.mult,
                )

                osb = opool.tile([HR, NHG * C], f32, tag="osb")
                for hg in range(NHG):
                    hT = hpsum.tile([F, HR], f32, tag="hT")
                    for b3 in range(HGB):
                        b = hg * HGB + b3
                        for q in range(NCHUNK):
                            tl = q * TPC + b
                            nc.tensor.matmul(
                                out=hT[:, b3 * R : (b3 + 1) * R],
                                lhsT=mb[:, tl * F : (tl + 1) * F],
                                rhs=s[:, (q * SPB + b) * R : (q * SPB + b + 1) * R],
                                start=(q == 0),
                                stop=False,
                            )
                        for q in range(NCHUNK):
                            tl = q * TPC + SPB
                            nc.tensor.matmul(
                                out=hT[:, b3 * R : (b3 + 1) * R],
                                lhsT=mb[:, tl * F : (tl + 1) * F],
                                rhs=s[
                                    :,
                                    GRID * R + q * SROWS + b * R :
                                    GRID * R + q * SROWS + (b + 1) * R,
                                ],
                                start=False,
                                stop=(q == NCHUNK - 1),
                            )
                    hts = htspool.tile([F, HR], bf16, tag="hts")
                    nc.scalar.activation(out=hts[:], in_=hT[:], func=Copy)
                    o_ps = opsum.tile([HR, C], f32, tag="o")
                    nc.tensor.matmul(
                        out=o_ps[:], lhsT=hts[:], rhs=wt_bf[:],
                        start=True, stop=True,
                    )
                    nc.vector.tensor_add(
                        out=osb[:, hg * C : (hg + 1) * C], in0=o_ps[:], in1=brep_ap
                    )
                ob = out_d[:]
                out_ap = AP(
                    ob.tensor,
                    sp * SROWS * C,
                    [[C, HR], [HR * C, NHG], [1, C]],
                )
                nc.sync.dma_start(out=out_ap, in_=osb[:])

    if not nc.is_finalized():
        nc.finalize()
    return nc


def _prep(feat, edge_row, edge_col, edge_val, W, b):
    """Host-side packing for the chunked-gather layout. See module docstring."""
    feat = np.ascontiguousarray(np.asarray(feat, dtype=np.float32))
    er = np.asarray(edge_row, dtype=np.int64)
    ec = np.asarray(edge_col, dtype=np.int64)
    ev = np.asarray(edge_val, dtype=np.float32)
    W = np.asarray(W, dtype=np.float32)
    b = np.asarray(b, dtype=np.float32)

    core = er // ROWS_PER_CORE          # 0..7
    lr = er % ROWS_PER_CORE             # core-local row
    blk = lr // R                       # block within core 0..419
    q = ec // CHUNK                     # chunk 0..3
    pk = (core * NB + blk) * NCHUNK + q
    order = np.lexsort((ec, pk))
    ecol = ec[order]
    evv = ev[order]
    pks = pk[order]
    lrs = lr[order]
    eqs = q[order]

    nbuk = N_CORES * NB * NCHUNK
    counts = np.bincount(pks, minlength=nbuk)
    starts = np.concatenate([[0], np.cumsum(counts)])
    rank = np.arange(N_EDGES) - starts[pks]

    cores = pks // (NB * NCHUNK)
    blks = (pks // NCHUNK) % NB
    sps = blks // SPB
    bls = blks % SPB

    GRID = NCHUNK * SPB
    NIDX = NSUP * NCHUNK * IDXC
    idx16 = np.zeros((N_CORES, 16, NIDX), dtype=np.int16)  # pad = chunk row 0
    lrow = np.full((N_CORES, P, NTILE), 1.0e6, dtype=np.float32)
    vals = np.zeros((N_CORES, P, NTILE), dtype=np.float32)

    grid = rank < P
    g_rank = rank[grid]
    slot = bls[grid] * P + g_rank
    colpos = (sps[grid] * NCHUNK + eqs[grid]) * IDXC + slot // 16
    idx16[cores[grid], slot % 16, colpos] = (
        ecol[grid] - eqs[grid] * CHUNK
    ).astype(np.int16)
    g_tile = sps[grid] * TPS + eqs[grid] * SPB + bls[grid]  # meta: grid-first
    lrow[cores[grid], g_rank, g_tile] = (lrs[grid] % R).astype(np.float32)
    vals[cores[grid], g_rank, g_tile] = evv[grid]

    # ---- overflow placement: per (core, super, chunk) shared tile ----
    ovf = ~grid
    if ovf.any():
        okey = (cores[ovf] * NSUP + sps[ovf]) * NCHUNK + eqs[ovf]
        oorder = np.argsort(okey, kind="stable")
        okey_s = okey[oorder]
        ocnt = np.bincount(okey_s, minlength=N_CORES * NSUP * NCHUNK)
        if ocnt.max() > P:
            raise RuntimeError(f"overflow tile overflow: {ocnt.max()} > {P}")
        ostarts = np.concatenate([[0], np.cumsum(ocnt)])
        okk = np.arange(okey_s.size) - ostarts[okey_s]
        oc = cores[ovf][oorder]
        osp = sps[ovf][oorder]
        oq = eqs[ovf][oorder]
        oslot = SPB * P + okk                     # within the (sp, q) call
        ocolpos = (osp * NCHUNK + oq) * IDXC + oslot // 16
        idx16[oc, oslot % 16, ocolpos] = (
            ecol[ovf][oorder] - oq * CHUNK
        ).astype(np.int16)
        o_tile = osp * TPS + GRID + oq            # meta: ov cols after grid
        lrow[oc, okk, o_tile] = (lrs[ovf][oorder] - osp * SROWS).astype(
            np.float32
        )
        vals[oc, okk, o_tile] = evv[ovf][oorder]

    iotar = np.tile(np.arange(R, dtype=np.float32)[None, :], (P, 1))
    iotas = np.tile(np.arange(SROWS, dtype=np.float32)[None, :], (P, 1))
    wt_pad = np.zeros((P, C), dtype=np.float32)
    wt_pad[:F, :] = W.T
    brep = np.tile(b[None, :], (P, 1)).astype(np.float32)

    in_maps = []
    for c in range(N_CORES):
        meta = np.concatenate(
            [lrow[c], vals[c], iotar, iotas, wt_pad, brep], axis=1
        )
        in_maps.append(
            {
                "feat": feat,
                "idx16": np.ascontiguousarray(np.tile(idx16[c], (8, 1))),
                "meta": np.ascontiguousarray(meta),
            }
        )
    return in_maps


def kernel(feat, edge_row, edge_col, edge_val, W, b, _trace=False, _trace_kwargs=None):
    from concourse.bass_utils import run_bass_kernel_spmd

    in_maps = _prep(feat, edge_row, edge_col, edge_val, W, b)
    if "prog" not in _CACHE:
        _CACHE["prog"] = _build_program()
    nc = _CACHE["prog"]
    kw = {}
    if _trace:
        kw["trace"] = True
        kw.update(_trace_kwargs or {})
    res = run_bass_kernel_spmd(nc, in_maps, list(range(N_CORES)), **kw)
    out = np.concatenate([r["out"][:ROWS_PER_CORE] for r in res.results], axis=0)
    if _trace:
        return out, res
    return out
